# revision 7
# baseline (speedup 1.0000x reference)
"""Trainium2 Bass kernel for EEGToLatentGCN.

Math: because the reference stacks all B*C nodes but uses a single 17-node
edge_index, message passing only ever touches global nodes 0..16 (= batch
element 0). Every other node goes through a plain per-node MLP:
    h = leaky(x @ We + be); h = leaky(h @ W1 + b1); h = leaky(h @ W2 + b2);
    h = leaky(h @ W3 + b3); g = mean_17(h); out = leaky(g @ Wh1 + bh1) @ Wh2 + bh2
The device kernel computes that MLP for all graphs (data-parallel over 8
cores); batch element 0 (17 nodes) is recomputed exactly on the host with the
real graph propagation and overwrites out[0].

Device layout (v2, engine-balanced, bf16):
- Host pre-transposes x to [81, R] bf16 (row 80 = ones) so the device DMAs
  x^T directly: no PE transposes, no staging copy, half the DMA bytes.
- b_emb/b1 ride as extra contraction rows (ones-row trick): embed K=81,
  conv1 K=65. Matmul cost is independent of K so the biases are free.
- All matmuls bf16 x bf16 -> f32 PSUM. Conv pairs write one 2-bank PSUM tile
  [128, 2, 512] so a pair's activation can be a single instruction.
- Activation work is balanced across three engines (an op may read only ONE
  PSUM operand, so PSUM-side leakys are either a 1-op ACT Lrelu or a DVE
  single-read bias-move followed by an SBUF-side leaky):
    ACT:  h0, h1 pair (merged, bias-free), h3c0, h2c0 every 3rd tile, head g1
    DVE:  bias-moves for h2c1/h3c1 (+h2c0 usually), 4x-mode bf16 leakys,
          head bias adds
    Pool: both 17-node sum-pools as 5-step scalar_tensor_tensor add-trees
- head: bh1 via ACT bias; bh2 via pre-broadcast [128, L] tensor_tensor adds
  on DVE (no PE bias matmuls). Head tiles are emitted with 2 macro-tiles of
  slack so PE never waits on freshly pooled sums.
"""

import numpy as np
import ml_dtypes

import concourse.bass as bass
import concourse.mybir as mybir
import concourse.tile as tile
from concourse import bacc
from concourse.bass_utils import run_bass_kernel_spmd

F32 = mybir.dt.float32
BF16 = mybir.dt.bfloat16
LRELU = mybir.ActivationFunctionType.Lrelu
ADD = mybir.AluOpType.add
MULT = mybir.AluOpType.mult
MAX = mybir.AluOpType.max
AX_X = mybir.AxisListType.X

NCORES = 8
B, C, T, F, H, L = 16384, 17, 80, 64, 256, 1024
BS = B // NCORES      # graphs per core
R = BS * C            # real node rows per core
MT_G = 30             # graphs per macro-tile
MT_R = MT_G * C       # 510 rows (fits a 2KB PSUM bank)
N_MT = 69             # ceil(BS / MT_G); last tile is 22 graphs of zero pad
G_PAD = N_MT * MT_G   # 2070
R_PAD = G_PAD * C     # 35190
HT_G = 256            # graphs per head tile
N_HT = BS // HT_G     # 8
XL_MT = 4             # macro-tiles per x DMA
HEAD_SLACK_G = 2 * MT_G  # delay head emission so pooled sums are ready
H2C0_ACT_EVERY = 3    # every Nth macro-tile h2c0 runs on ACT instead of DVE
SLOPE = 0.01

_CACHE = {}


def _leaky_np(v):
    return np.where(v > 0, v, SLOPE * v)


def _build(reps=1):
    nc = bacc.Bacc("TRN2", target_bir_lowering=False, debug=False)

    xt_p = nc.declare_dram_parameter("xt", [T + 1, R_PAD], BF16, isOutput=False)
    wembx_p = nc.declare_dram_parameter("wembx", [T + 1, F], BF16, isOutput=False)
    w1x_p = nc.declare_dram_parameter("w1x", [F + 1, H], BF16, isOutput=False)
    w2_p = nc.declare_dram_parameter("w2", [128, 2, H], BF16, isOutput=False)
    b2_p = nc.declare_dram_parameter("b2", [128, 2], F32, isOutput=False)
    w3_p = nc.declare_dram_parameter("w3", [128, 2, H], BF16, isOutput=False)
    b3_p = nc.declare_dram_parameter("b3", [128, 2], F32, isOutput=False)
    wh1_p = nc.declare_dram_parameter("wh1", [128, 2, H], BF16, isOutput=False)
    bh1_p = nc.declare_dram_parameter("bh1", [128, 2], F32, isOutput=False)
    wh2_p = nc.declare_dram_parameter("wh2", [128, 2, L], BF16, isOutput=False)
    bh2b_p = nc.declare_dram_parameter("bh2b", [128, L], F32, isOutput=False)
    out_p = nc.declare_dram_parameter("out", [BS, L], F32, isOutput=True)

    with tile.TileContext(nc) as tc:
        with tc.tile_pool(name="consts", bufs=1) as consts:
            wembx_t = consts.tile([T + 1, F], BF16)
            w1x_t = consts.tile([F + 1, H], BF16)
            w2_t = consts.tile([128, 2, H], BF16)
            b2_t = consts.tile([128, 2], F32)
            w3_t = consts.tile([128, 2, H], BF16)
            b3_t = consts.tile([128, 2], F32)
            wh1_t = consts.tile([128, 2, H], BF16)
            bh1_t = consts.tile([128, 2], F32)
            wh2_t = consts.tile([128, 2, L], BF16)
            bh2b_t = consts.tile([128, L], F32)
            for dst_t, src_p in [
                (wembx_t, wembx_p), (w1x_t, w1x_p),
                (w2_t, w2_p), (b2_t, b2_p), (w3_t, w3_p), (b3_t, b3_p),
                (wh1_t, wh1_p), (bh1_t, bh1_p),
                (wh2_t, wh2_p), (bh2b_t, bh2b_p),
            ]:
                nc.sync.dma_start(dst_t[:], src_p[:])

            # pooled per-graph sums (head input), persistent
            gt = consts.tile([128, 2, G_PAD], BF16)

            # h0 tiles carry a constant-ones row 64 so conv1's bias rides the
            # matmul; manual 3-buffer rotation keeps the row intact.
            ones_row = consts.tile([1, MT_R], F32)
            nc.vector.memset(ones_row[:], 1.0)
            h0s = []
            for i in range(3):
                h0buf = consts.tile([F + 1, MT_R], BF16, name=f"h0_{i}")
                nc.vector.tensor_copy(h0buf[F:F + 1, :], ones_row[:])
                h0s.append(h0buf)

            for _rep in range(reps):
              with tc.tile_pool(name="xl", bufs=2) as xlp, \
                 tc.tile_pool(name="hw", bufs=3) as hw, \
                 tc.tile_pool(name="hd", bufs=2) as hd, \
                 tc.tile_pool(name="scr", bufs=2) as scr, \
                 tc.tile_pool(name="ps0", bufs=1, space="PSUM") as ps0p, \
                 tc.tile_pool(name="pspair", bufs=2, space="PSUM") as pppool, \
                 tc.tile_pool(name="psg1", bufs=1, space="PSUM") as psg1p, \
                 tc.tile_pool(name="pso", bufs=2, space="PSUM") as psop:

                def emit_head(ht):
                    g0 = ht * HT_G
                    g1 = hd.tile([128, 2, HT_G], BF16, tag="g1")
                    for m in range(2):
                        psg1 = psg1p.tile([128, HT_G], F32, tag="psg1")
                        nc.tensor.matmul(psg1[:],
                                         wh1_t[:, 0, m * 128:(m + 1) * 128],
                                         gt[:, 0, g0:g0 + HT_G],
                                         start=True, stop=False)
                        nc.tensor.matmul(psg1[:],
                                         wh1_t[:, 1, m * 128:(m + 1) * 128],
                                         gt[:, 1, g0:g0 + HT_G],
                                         start=False, stop=True)
                        nc.scalar.activation(g1[:, m, :], psg1[:], LRELU,
                                             bias=bh1_t[:, m:m + 1], scale=1.0,
                                             alpha=SLOPE)
                    for m in range(2):
                        o_sb = hd.tile([128, L], F32, tag="osb")
                        for nb in range(2):
                            pso = psop.tile([128, 512], F32, tag="pso")
                            nc.tensor.matmul(
                                pso[:],
                                g1[:, 0, m * 128:(m + 1) * 128],
                                wh2_t[:, 0, nb * 512:(nb + 1) * 512],
                                start=True, stop=False)
                            nc.tensor.matmul(
                                pso[:],
                                g1[:, 1, m * 128:(m + 1) * 128],
                                wh2_t[:, 1, nb * 512:(nb + 1) * 512],
                                start=False, stop=True)
                            nc.vector.tensor_tensor(
                                o_sb[:, nb * 512:(nb + 1) * 512], pso[:],
                                bh2b_t[:, nb * 512:(nb + 1) * 512], op=ADD)
                        nc.sync.dma_start(
                            out_p[g0 + m * 128:g0 + (m + 1) * 128, :], o_sb[:])

                def dve_chunk(h_t, pp_t, c, b_t, utag):
                    # single-PSUM-read bias move, then 4x-mode bf16 leaky
                    u = scr.tile([128, MT_R], BF16, tag=utag)
                    nc.vector.tensor_scalar(u[:], pp_t[:, c, 0:MT_R],
                                            b_t[:, c:c + 1], None, op0=ADD)
                    nc.vector.scalar_tensor_tensor(
                        h_t[:, c, :], u[:], SLOPE, u[:], op0=MULT, op1=MAX)

                def pool_tree(h_t, c, g0):
                    # 17-node sum pool on GPSIMD: 8+8+1 add tree, f32 scratch
                    h3v = h_t[:, c, :].rearrange("p (g s) -> p g s", s=C)
                    sc = scr.tile([128, MT_G, 8], F32, tag=f"sc{c}")
                    sc2 = scr.tile([128, MT_G, 4], F32, tag=f"sc2_{c}")
                    gp = nc.gpsimd
                    gp.tensor_tensor(sc[:, :, :], h3v[:, :, 0:8],
                                     h3v[:, :, 8:16], op=ADD)
                    gp.tensor_tensor(sc2[:, :, :], sc[:, :, 0:4],
                                     sc[:, :, 4:8], op=ADD)
                    gp.tensor_tensor(sc[:, :, 0:2], sc2[:, :, 0:2],
                                     sc2[:, :, 2:4], op=ADD)
                    gp.tensor_tensor(sc2[:, :, 0:1], sc[:, :, 0:1],
                                     sc[:, :, 1:2], op=ADD)
                    gp.tensor_tensor(
                        gt[:, c, g0:g0 + MT_G],
                        sc2[:, :, 0:1].rearrange("p g o -> p (g o)"),
                        h3v[:, :, 16:17].rearrange("p g o -> p (g o)"),
                        op=ADD)

                next_ht = 0
                xl = None
                for mt in range(N_MT):
                    j = mt % XL_MT
                    if j == 0:
                        cols = min(XL_MT * MT_R, R_PAD - mt * MT_R)
                        xl = xlp.tile([T + 1, XL_MT * MT_R], BF16, tag="xl")
                        nc.sync.dma_start(
                            xl[:, 0:cols],
                            xt_p[:, mt * MT_R:mt * MT_R + cols])
                    xs = xl[:, j * MT_R:(j + 1) * MT_R]

                    # embed [81]->[64] (bias via ones row), leaky on ACT
                    ps0 = ps0p.tile([F, MT_R], F32, tag="ps0")
                    nc.tensor.matmul(ps0[:], wembx_t[:], xs,
                                     start=True, stop=True)
                    h0 = h0s[mt % 3]
                    nc.scalar.activation(h0[0:F, :], ps0[:], LRELU,
                                         bias=0.0, scale=1.0, alpha=SLOPE)

                    # conv1 [65]->[256] (bias via h0 ones row), merged pair
                    # leaky on ACT
                    pp1 = pppool.tile([128, 2, 512], F32, tag="pp")
                    for c in range(2):
                        nc.tensor.matmul(pp1[:, c, 0:MT_R],
                                         w1x_t[:, c * 128:(c + 1) * 128],
                                         h0[:], start=True, stop=True)
                    h1 = hw.tile([128, 2, MT_R], BF16, tag="h1")
                    nc.scalar.activation(h1[:, :, :], pp1[:, :, 0:MT_R],
                                         LRELU, bias=0.0, scale=1.0,
                                         alpha=SLOPE)

                    # conv2 [256]->[256]
                    pp2 = pppool.tile([128, 2, 512], F32, tag="pp")
                    for c in range(2):
                        nc.tensor.matmul(pp2[:, c, 0:MT_R],
                                         w2_t[:, 0, c * 128:(c + 1) * 128],
                                         h1[:, 0, :], start=True, stop=False)
                        nc.tensor.matmul(pp2[:, c, 0:MT_R],
                                         w2_t[:, 1, c * 128:(c + 1) * 128],
                                         h1[:, 1, :], start=False, stop=True)
                    h2 = hw.tile([128, 2, MT_R], BF16, tag="h2")
                    if mt % 2 == 0:
                        nc.scalar.activation(h2[:, 0, :], pp2[:, 0, 0:MT_R],
                                             LRELU, bias=b2_t[:, 0:1],
                                             scale=1.0, alpha=SLOPE)
                    else:
                        dve_chunk(h2, pp2, 0, b2_t, "u20")
                    dve_chunk(h2, pp2, 1, b2_t, "u21")

                    # conv3 [256]->[256]
                    pp3 = pppool.tile([128, 2, 512], F32, tag="pp")
                    for c in range(2):
                        nc.tensor.matmul(pp3[:, c, 0:MT_R],
                                         w3_t[:, 0, c * 128:(c + 1) * 128],
                                         h2[:, 0, :], start=True, stop=False)
                        nc.tensor.matmul(pp3[:, c, 0:MT_R],
                                         w3_t[:, 1, c * 128:(c + 1) * 128],
                                         h2[:, 1, :], start=False, stop=True)
                    h3 = hw.tile([128, 2, MT_R], BF16, tag="h3")
                    nc.scalar.activation(h3[:, 0, :], pp3[:, 0, 0:MT_R],
                                         LRELU, bias=b3_t[:, 0:1],
                                         scale=1.0, alpha=SLOPE)
                    dve_chunk(h3, pp3, 1, b3_t, "u31")

                    # 17-node sum pool (1/17 folded into Wh1): chunk 1 always
                    # a GPSIMD add-tree; chunk 0 alternates DVE tensor_reduce
                    # and a second GPSIMD tree to balance engine load
                    g0 = mt * MT_G
                    with nc.allow_low_precision(
                            reason="pooled sums rounded to bf16 for the "
                                   "bf16 head matmul"):
                        if mt % 2 == 0:
                            nc.vector.tensor_reduce(
                                out=gt[:, 0, g0:g0 + MT_G],
                                in_=h3[:, 0, :].rearrange(
                                    "p (g s) -> p g s", s=C),
                                op=ADD, axis=AX_X)
                        else:
                            pool_tree(h3, 0, g0)
                        pool_tree(h3, 1, g0)

                    done = (mt + 1) * MT_G - HEAD_SLACK_G
                    while (next_ht < N_HT
                           and (next_ht + 1) * HT_G <= done):
                        emit_head(next_ht)
                        next_ht += 1

                while next_ht < N_HT:
                    emit_head(next_ht)
                    next_ht += 1

    nc.compile()
    return nc


def _get_nc(reps=1):
    key = ("nc", reps)
    if key not in _CACHE:
        _CACHE[key] = _build(reps)
    return _CACHE[key]


def prep_in_maps(x, W_emb, b_emb, W1, b1, W2, b2, W3, b3, Wh1, bh1, Wh2, bh2):
    """Host-side layout prep: per-core input maps for run_bass_kernel_spmd."""
    bf16 = ml_dtypes.bfloat16

    def kchunks(w):
        # [256, out] -> [128, 2, out] (k-chunk as middle axis)
        return np.ascontiguousarray(
            w.reshape(2, 128, w.shape[1]).transpose(1, 0, 2)).astype(bf16)

    def bcols(b):
        # [256] -> [128, 2]
        return np.ascontiguousarray(b.reshape(2, 128).T)

    weights = {
        "wembx": np.ascontiguousarray(
            np.concatenate([W_emb, b_emb[None, :]], axis=0)).astype(bf16),
        "w1x": np.ascontiguousarray(
            np.concatenate([W1, b1[None, :]], axis=0)).astype(bf16),
        "w2": kchunks(W2), "b2": bcols(b2),
        "w3": kchunks(W3), "b3": bcols(b3),
        "wh1": kchunks(Wh1 * (1.0 / C)), "bh1": bcols(bh1),
        "wh2": kchunks(Wh2),
        "bh2b": np.ascontiguousarray(
            np.broadcast_to(bh2[None, :], (128, L)).copy()),
    }

    xr = x.reshape(B * C, T)
    in_maps = []
    for i in range(NCORES):
        xt = np.zeros((T + 1, R_PAD), np.float32)
        xt[0:T, 0:R] = xr[i * R:(i + 1) * R].T
        xt[T, :] = 1.0
        m = dict(weights)
        m["xt"] = xt.astype(bf16)
        in_maps.append(m)
    return in_maps


def _fixup_graph0(x, W_emb, b_emb, W1, b1, W2, b2, W3, b3, Wh1, bh1, Wh2, bh2,
                  src, dst):
    """Exact recompute of batch element 0 with real GCN propagation."""
    deg = np.ones(C, np.float64)
    np.add.at(deg, dst.astype(np.int64), 1.0)
    dinv = 1.0 / np.sqrt(deg)
    A = np.zeros((C, C), np.float64)
    A[np.arange(C), np.arange(C)] = dinv * dinv
    np.add.at(A, (dst.astype(np.int64), src.astype(np.int64)),
              dinv[src.astype(np.int64)] * dinv[dst.astype(np.int64)])

    h = _leaky_np(x[0].astype(np.float64) @ W_emb + b_emb)
    for Wc, bc in [(W1, b1), (W2, b2), (W3, b3)]:
        h = _leaky_np(A @ (h @ Wc) + bc)
    g = h.mean(axis=0)
    return (_leaky_np(g @ Wh1 + bh1) @ Wh2 + bh2).astype(np.float32)


def kernel(x, W_emb, b_emb, W1, b1, W2, b2, W3, b3, Wh1, bh1, Wh2, bh2,
           src, dst):
    x = np.ascontiguousarray(np.asarray(x, np.float32))
    W_emb = np.asarray(W_emb, np.float32)
    b_emb = np.asarray(b_emb, np.float32)
    W1 = np.asarray(W1, np.float32)
    b1 = np.asarray(b1, np.float32)
    W2 = np.asarray(W2, np.float32)
    b2 = np.asarray(b2, np.float32)
    W3 = np.asarray(W3, np.float32)
    b3 = np.asarray(b3, np.float32)
    Wh1 = np.asarray(Wh1, np.float32)
    bh1 = np.asarray(bh1, np.float32)
    Wh2 = np.asarray(Wh2, np.float32)
    bh2 = np.asarray(bh2, np.float32)

    in_maps = prep_in_maps(x, W_emb, b_emb, W1, b1, W2, b2, W3, b3,
                           Wh1, bh1, Wh2, bh2)

    nc = _get_nc()
    res = run_bass_kernel_spmd(nc, in_maps, core_ids=list(range(NCORES)))
    out = np.concatenate([res.results[i]["out"] for i in range(NCORES)], axis=0)

    out[0] = _fixup_graph0(x, W_emb, b_emb, W1, b1, W2, b2, W3, b3,
                           Wh1, bh1, Wh2, bh2, np.asarray(src), np.asarray(dst))
    return out


# revision 13
# speedup vs baseline: 1.0287x; 1.0287x over previous
"""Trainium2 Bass kernel for EEGToLatentGCN.

Math: because the reference stacks all B*C nodes but uses a single 17-node
edge_index, message passing only ever touches global nodes 0..16 (= batch
element 0). Every other node goes through a plain per-node MLP:
    h = leaky(x @ We + be); h = leaky(h @ W1 + b1); h = leaky(h @ W2 + b2);
    h = leaky(h @ W3 + b3); g = mean_17(h); out = leaky(g @ Wh1 + bh1) @ Wh2 + bh2
The device kernel computes that MLP for all graphs (data-parallel over 8
cores); batch element 0 (17 nodes) is recomputed exactly on the host with the
real graph propagation and overwrites out[0].

Device layout (v2, engine-balanced, bf16):
- Host pre-transposes x to [81, R] bf16 (row 80 = ones) so the device DMAs
  x^T directly: no PE transposes, no staging copy, half the DMA bytes.
- b_emb/b1 ride as extra contraction rows (ones-row trick): embed K=81,
  conv1 K=65. Matmul cost is independent of K so the biases are free.
- All matmuls bf16 x bf16 -> f32 PSUM. Conv pairs write one 2-bank PSUM tile
  [128, 2, 512] so a pair's activation can be a single instruction.
- Activation work is balanced across three engines (an op may read only ONE
  PSUM operand, so PSUM-side leakys are either a 1-op ACT Lrelu or a DVE
  single-read bias-move followed by an SBUF-side leaky):
    ACT:  h0, h1 pair (merged, bias-free), h3c0, h2c0 every 3rd tile, head g1
    DVE:  bias-moves for h2c1/h3c1 (+h2c0 usually), 4x-mode bf16 leakys,
          head bias adds
    Pool: both 17-node sum-pools as 5-step scalar_tensor_tensor add-trees
- head: bh1 via ACT bias; bh2 via pre-broadcast [128, L] tensor_tensor adds
  on DVE (no PE bias matmuls). Head tiles are emitted with 2 macro-tiles of
  slack so PE never waits on freshly pooled sums.
"""

import numpy as np
import ml_dtypes

import concourse.bass as bass
import concourse.mybir as mybir
import concourse.tile as tile
from concourse import bacc
from concourse.bass_utils import run_bass_kernel_spmd

F32 = mybir.dt.float32
BF16 = mybir.dt.bfloat16
LRELU = mybir.ActivationFunctionType.Lrelu
ADD = mybir.AluOpType.add
MULT = mybir.AluOpType.mult
MAX = mybir.AluOpType.max
AX_X = mybir.AxisListType.X

NCORES = 8
B, C, T, F, H, L = 16384, 17, 80, 64, 256, 1024
BS = B // NCORES      # graphs per core
R = BS * C            # real node rows per core
MT_G = 30             # graphs per macro-tile
MT_R = MT_G * C       # 510 rows (fits a 2KB PSUM bank)
N_MT = 69             # ceil(BS / MT_G); last tile is 22 graphs of zero pad
G_PAD = N_MT * MT_G   # 2070
R_PAD = G_PAD * C     # 35190
HT_G = 256            # graphs per head tile
N_HT = BS // HT_G     # 8
XL_MT = 4             # macro-tiles per x DMA
HEAD_SLACK_G = 2 * MT_G  # delay head emission so pooled sums are ready
H2C0_ACT_EVERY = 3    # every Nth macro-tile h2c0 runs on ACT instead of DVE
SLOPE = 0.01

_CACHE = {}


def _leaky_np(v):
    return np.where(v > 0, v, SLOPE * v)


def _build(reps=1):
    nc = bacc.Bacc("TRN2", target_bir_lowering=False, debug=False)

    xt_p = nc.declare_dram_parameter("xt", [T + 1, R_PAD], BF16, isOutput=False)
    wembx_p = nc.declare_dram_parameter("wembx", [T + 1, F], BF16, isOutput=False)
    w1x_p = nc.declare_dram_parameter("w1x", [F + 1, H], BF16, isOutput=False)
    w2_p = nc.declare_dram_parameter("w2", [128, 2, H], BF16, isOutput=False)
    b2_p = nc.declare_dram_parameter("b2", [128, 2], F32, isOutput=False)
    w3_p = nc.declare_dram_parameter("w3", [128, 2, H], BF16, isOutput=False)
    b3_p = nc.declare_dram_parameter("b3", [128, 2], F32, isOutput=False)
    wh1_p = nc.declare_dram_parameter("wh1", [128, 2, H], BF16, isOutput=False)
    bh1_p = nc.declare_dram_parameter("bh1", [128, 2], F32, isOutput=False)
    wh2_p = nc.declare_dram_parameter("wh2", [128, 2, L], BF16, isOutput=False)
    bh2b_p = nc.declare_dram_parameter("bh2b", [128, L], F32, isOutput=False)
    out_p = nc.declare_dram_parameter("out", [BS, L], F32, isOutput=True)

    with tile.TileContext(nc) as tc:
        with tc.tile_pool(name="consts", bufs=1) as consts:
            wembx_t = consts.tile([T + 1, F], BF16)
            w1x_t = consts.tile([F + 1, H], BF16)
            w2_t = consts.tile([128, 2, H], BF16)
            b2_t = consts.tile([128, 2], F32)
            w3_t = consts.tile([128, 2, H], BF16)
            b3_t = consts.tile([128, 2], F32)
            wh1_t = consts.tile([128, 2, H], BF16)
            bh1_t = consts.tile([128, 2], F32)
            wh2_t = consts.tile([128, 2, L], BF16)
            bh2b_t = consts.tile([128, L], F32)
            for dst_t, src_p in [
                (wembx_t, wembx_p), (w1x_t, w1x_p),
                (w2_t, w2_p), (b2_t, b2_p), (w3_t, w3_p), (b3_t, b3_p),
                (wh1_t, wh1_p), (bh1_t, bh1_p),
                (wh2_t, wh2_p), (bh2b_t, bh2b_p),
            ]:
                nc.sync.dma_start(dst_t[:], src_p[:])

            # pooled per-graph sums (head input), persistent
            gt = consts.tile([128, 2, G_PAD], BF16)

            # constant 0.01 tile: Pool-side leaky slope operand
            c001 = consts.tile([128, MT_R], BF16)
            nc.vector.memset(c001[:], SLOPE)

            # h0 tiles carry a constant-ones row 64 so conv1's bias rides the
            # matmul; manual 3-buffer rotation keeps the row intact.
            ones_row = consts.tile([1, MT_R], F32)
            nc.vector.memset(ones_row[:], 1.0)
            h0s = []
            for i in range(3):
                h0buf = consts.tile([F + 1, MT_R], BF16, name=f"h0_{i}")
                nc.vector.tensor_copy(h0buf[F:F + 1, :], ones_row[:])
                h0s.append(h0buf)

            for _rep in range(reps):
              with tc.tile_pool(name="xl", bufs=2) as xlp, \
                 tc.tile_pool(name="hw", bufs=3) as hw, \
                 tc.tile_pool(name="hd", bufs=2) as hd, \
                 tc.tile_pool(name="scr", bufs=2) as scr, \
                 tc.tile_pool(name="ps0", bufs=1, space="PSUM") as ps0p, \
                 tc.tile_pool(name="pspair", bufs=2, space="PSUM") as pppool, \
                 tc.tile_pool(name="psg1", bufs=1, space="PSUM") as psg1p, \
                 tc.tile_pool(name="pso", bufs=2, space="PSUM") as psop:

                def emit_head(ht):
                    g0 = ht * HT_G
                    g1 = hd.tile([128, 2, HT_G], BF16, tag="g1")
                    for m in range(2):
                        psg1 = psg1p.tile([128, HT_G], F32, tag="psg1")
                        nc.tensor.matmul(psg1[:],
                                         wh1_t[:, 0, m * 128:(m + 1) * 128],
                                         gt[:, 0, g0:g0 + HT_G],
                                         start=True, stop=False)
                        nc.tensor.matmul(psg1[:],
                                         wh1_t[:, 1, m * 128:(m + 1) * 128],
                                         gt[:, 1, g0:g0 + HT_G],
                                         start=False, stop=True)
                        nc.scalar.activation(g1[:, m, :], psg1[:], LRELU,
                                             bias=bh1_t[:, m:m + 1], scale=1.0,
                                             alpha=SLOPE)
                    for m in range(2):
                        o_sb = hd.tile([128, L], F32, tag="osb")
                        for nb in range(2):
                            pso = psop.tile([128, 512], F32, tag="pso")
                            nc.tensor.matmul(
                                pso[:],
                                g1[:, 0, m * 128:(m + 1) * 128],
                                wh2_t[:, 0, nb * 512:(nb + 1) * 512],
                                start=True, stop=False)
                            nc.tensor.matmul(
                                pso[:],
                                g1[:, 1, m * 128:(m + 1) * 128],
                                wh2_t[:, 1, nb * 512:(nb + 1) * 512],
                                start=False, stop=True)
                            nc.vector.tensor_tensor(
                                o_sb[:, nb * 512:(nb + 1) * 512], pso[:],
                                bh2b_t[:, nb * 512:(nb + 1) * 512], op=ADD)
                        nc.sync.dma_start(
                            out_p[g0 + m * 128:g0 + (m + 1) * 128, :], o_sb[:])

                def dve_move(pp_t, c, b_t, utag):
                    # single-PSUM-read bias add into bf16 SBUF
                    u = scr.tile([128, MT_R], BF16, tag=utag, name="u")
                    nc.vector.tensor_scalar(u[:], pp_t[:, c, 0:MT_R],
                                            b_t[:, c:c + 1], None, op0=ADD)
                    return u

                next_ht = 0
                xl = None
                for mt in range(N_MT):
                    j = mt % XL_MT
                    if j == 0:
                        cols = min(XL_MT * MT_R, R_PAD - mt * MT_R)
                        xl = xlp.tile([T + 1, XL_MT * MT_R], BF16, tag="xl")
                        nc.sync.dma_start(
                            xl[:, 0:cols],
                            xt_p[:, mt * MT_R:mt * MT_R + cols])
                    xs = xl[:, j * MT_R:(j + 1) * MT_R]

                    # embed [81]->[64] (bias via ones row), leaky on ACT
                    ps0 = ps0p.tile([F, MT_R], F32, tag="ps0")
                    nc.tensor.matmul(ps0[:], wembx_t[:], xs,
                                     start=True, stop=True)
                    h0 = h0s[mt % 3]
                    nc.scalar.activation(h0[0:F, :], ps0[:], LRELU,
                                         bias=0.0, scale=1.0, alpha=SLOPE)

                    # conv1 [65]->[256] (bias via h0 ones row), merged pair
                    # leaky on ACT
                    pp1 = pppool.tile([128, 2, 512], F32, tag="pp")
                    for c in range(2):
                        nc.tensor.matmul(pp1[:, c, 0:MT_R],
                                         w1x_t[:, c * 128:(c + 1) * 128],
                                         h0[:], start=True, stop=True)
                    h1 = hw.tile([128, 2, MT_R], BF16, tag="h1")
                    nc.scalar.activation(h1[:, :, :], pp1[:, :, 0:MT_R],
                                         LRELU, bias=0.0, scale=1.0,
                                         alpha=SLOPE)

                    # conv2 [256]->[256]
                    pp2 = pppool.tile([128, 2, 512], F32, tag="pp")
                    for c in range(2):
                        nc.tensor.matmul(pp2[:, c, 0:MT_R],
                                         w2_t[:, 0, c * 128:(c + 1) * 128],
                                         h1[:, 0, :], start=True, stop=False)
                        nc.tensor.matmul(pp2[:, c, 0:MT_R],
                                         w2_t[:, 1, c * 128:(c + 1) * 128],
                                         h1[:, 1, :], start=False, stop=True)
                    h2 = hw.tile([128, 2, MT_R], BF16, tag="h2")
                    nc.scalar.activation(h2[:, 0, :], pp2[:, 0, 0:MT_R],
                                         LRELU, bias=b2_t[:, 0:1],
                                         scale=1.0, alpha=SLOPE)
                    # c1 leaky: DVE bias-move, slope-mult on Pool, max on DVE
                    u2 = dve_move(pp2, 1, b2_t, "u2")
                    v2 = scr.tile([128, MT_R], BF16, tag="v2")
                    nc.gpsimd.tensor_tensor(v2[:], u2[:], c001[:], op=MULT)
                    nc.vector.tensor_tensor(h2[:, 1, :], u2[:], v2[:], op=MAX)

                    # conv3 [256]->[256]
                    pp3 = pppool.tile([128, 2, 512], F32, tag="pp")
                    for c in range(2):
                        nc.tensor.matmul(pp3[:, c, 0:MT_R],
                                         w3_t[:, 0, c * 128:(c + 1) * 128],
                                         h2[:, 0, :], start=True, stop=False)
                        nc.tensor.matmul(pp3[:, c, 0:MT_R],
                                         w3_t[:, 1, c * 128:(c + 1) * 128],
                                         h2[:, 1, :], start=False, stop=True)
                    h3 = hw.tile([128, 2, MT_R], BF16, tag="h3")
                    nc.scalar.activation(h3[:, 0, :], pp3[:, 0, 0:MT_R],
                                         LRELU, bias=b3_t[:, 0:1],
                                         scale=1.0, alpha=SLOPE)
                    # c1 leaky: DVE bias-move, slope-mult on Pool, max on DVE
                    u3 = dve_move(pp3, 1, b3_t, "u3")
                    v3 = scr.tile([128, MT_R], BF16, tag="v3")
                    nc.gpsimd.tensor_tensor(v3[:], u3[:], c001[:], op=MULT)
                    nc.vector.tensor_tensor(h3[:, 1, :], u3[:], v3[:], op=MAX)

                    # 17-node sum pool (1/17 folded into Wh1): Pool halves
                    # nodes 0..15, DVE reduces the 8 sums, adds node 16
                    g0 = mt * MT_G
                    h3v = h3[:, :, :].rearrange("p c (g s) -> p c g s", s=C)
                    sc8 = scr.tile([128, 2, MT_G, 8], F32, tag="sc8")
                    tmp = scr.tile([128, 2, MT_G], F32, tag="tmp")
                    nc.gpsimd.tensor_tensor(sc8[:, :, :, :],
                                            h3v[:, :, :, 0:8],
                                            h3v[:, :, :, 8:16], op=ADD)
                    with nc.allow_low_precision(
                            reason="pooled sums rounded to bf16 for the "
                                   "bf16 head matmul"):
                        nc.vector.tensor_reduce(
                            out=tmp[:, :, :], in_=sc8[:, :, :, :],
                            op=ADD, axis=AX_X)
                        nc.vector.tensor_tensor(
                            gt[:, :, g0:g0 + MT_G], tmp[:, :, :],
                            h3v[:, :, :, 16:17].rearrange(
                                "p c g o -> p c (g o)"), op=ADD)

                    done = (mt + 1) * MT_G - HEAD_SLACK_G
                    while (next_ht < N_HT
                           and (next_ht + 1) * HT_G <= done):
                        emit_head(next_ht)
                        next_ht += 1

                while next_ht < N_HT:
                    emit_head(next_ht)
                    next_ht += 1

    nc.compile()
    return nc


def _get_nc(reps=1):
    key = ("nc", reps)
    if key not in _CACHE:
        _CACHE[key] = _build(reps)
    return _CACHE[key]


def prep_in_maps(x, W_emb, b_emb, W1, b1, W2, b2, W3, b3, Wh1, bh1, Wh2, bh2):
    """Host-side layout prep: per-core input maps for run_bass_kernel_spmd."""
    bf16 = ml_dtypes.bfloat16

    def kchunks(w):
        # [256, out] -> [128, 2, out] (k-chunk as middle axis)
        return np.ascontiguousarray(
            w.reshape(2, 128, w.shape[1]).transpose(1, 0, 2)).astype(bf16)

    def bcols(b):
        # [256] -> [128, 2]
        return np.ascontiguousarray(b.reshape(2, 128).T)

    weights = {
        "wembx": np.ascontiguousarray(
            np.concatenate([W_emb, b_emb[None, :]], axis=0)).astype(bf16),
        "w1x": np.ascontiguousarray(
            np.concatenate([W1, b1[None, :]], axis=0)).astype(bf16),
        "w2": kchunks(W2), "b2": bcols(b2),
        "w3": kchunks(W3), "b3": bcols(b3),
        "wh1": kchunks(Wh1 * (1.0 / C)), "bh1": bcols(bh1),
        "wh2": kchunks(Wh2),
        "bh2b": np.ascontiguousarray(
            np.broadcast_to(bh2[None, :], (128, L)).copy()),
    }

    xr = x.reshape(B * C, T)
    in_maps = []
    for i in range(NCORES):
        xt = np.zeros((T + 1, R_PAD), np.float32)
        xt[0:T, 0:R] = xr[i * R:(i + 1) * R].T
        xt[T, :] = 1.0
        m = dict(weights)
        m["xt"] = xt.astype(bf16)
        in_maps.append(m)
    return in_maps


def _fixup_graph0(x, W_emb, b_emb, W1, b1, W2, b2, W3, b3, Wh1, bh1, Wh2, bh2,
                  src, dst):
    """Exact recompute of batch element 0 with real GCN propagation."""
    deg = np.ones(C, np.float64)
    np.add.at(deg, dst.astype(np.int64), 1.0)
    dinv = 1.0 / np.sqrt(deg)
    A = np.zeros((C, C), np.float64)
    A[np.arange(C), np.arange(C)] = dinv * dinv
    np.add.at(A, (dst.astype(np.int64), src.astype(np.int64)),
              dinv[src.astype(np.int64)] * dinv[dst.astype(np.int64)])

    h = _leaky_np(x[0].astype(np.float64) @ W_emb + b_emb)
    for Wc, bc in [(W1, b1), (W2, b2), (W3, b3)]:
        h = _leaky_np(A @ (h @ Wc) + bc)
    g = h.mean(axis=0)
    return (_leaky_np(g @ Wh1 + bh1) @ Wh2 + bh2).astype(np.float32)


def kernel(x, W_emb, b_emb, W1, b1, W2, b2, W3, b3, Wh1, bh1, Wh2, bh2,
           src, dst):
    x = np.ascontiguousarray(np.asarray(x, np.float32))
    W_emb = np.asarray(W_emb, np.float32)
    b_emb = np.asarray(b_emb, np.float32)
    W1 = np.asarray(W1, np.float32)
    b1 = np.asarray(b1, np.float32)
    W2 = np.asarray(W2, np.float32)
    b2 = np.asarray(b2, np.float32)
    W3 = np.asarray(W3, np.float32)
    b3 = np.asarray(b3, np.float32)
    Wh1 = np.asarray(Wh1, np.float32)
    bh1 = np.asarray(bh1, np.float32)
    Wh2 = np.asarray(Wh2, np.float32)
    bh2 = np.asarray(bh2, np.float32)

    in_maps = prep_in_maps(x, W_emb, b_emb, W1, b1, W2, b2, W3, b3,
                           Wh1, bh1, Wh2, bh2)

    nc = _get_nc()
    res = run_bass_kernel_spmd(nc, in_maps, core_ids=list(range(NCORES)))
    out = np.concatenate([res.results[i]["out"] for i in range(NCORES)], axis=0)

    out[0] = _fixup_graph0(x, W_emb, b_emb, W1, b1, W2, b2, W3, b3,
                           Wh1, bh1, Wh2, bh2, np.asarray(src), np.asarray(dst))
    return out


# revision 18
# speedup vs baseline: 1.2310x; 1.1967x over previous
"""Trainium2 Bass kernel for EEGToLatentGCN.

Math: because the reference stacks all B*C nodes but uses a single 17-node
edge_index, message passing only ever touches global nodes 0..16 (= batch
element 0). Every other node goes through a plain per-node MLP:
    h = leaky(x @ We + be); h = leaky(h @ W1 + b1); h = leaky(h @ W2 + b2);
    h = leaky(h @ W3 + b3); g = mean_17(h); out = leaky(g @ Wh1 + bh1) @ Wh2 + bh2
The device kernel computes that MLP for all graphs (data-parallel over 8
cores); batch element 0 (17 nodes) is recomputed exactly on the host with the
real graph propagation and overwrites out[0].

Device layout (v2, engine-balanced, bf16):
- Host pre-transposes x to [81, R] bf16 (row 80 = ones) so the device DMAs
  x^T directly: no PE transposes, no staging copy, half the DMA bytes.
- b_emb/b1 ride as extra contraction rows (ones-row trick): embed K=81,
  conv1 K=65. Matmul cost is independent of K so the biases are free.
- All matmuls bf16 x bf16 -> f32 PSUM. Conv pairs write one 2-bank PSUM tile
  [128, 2, 512] so a pair's activation can be a single instruction.
- Activation work is balanced across three engines (an op may read only ONE
  PSUM operand, so PSUM-side leakys are either a 1-op ACT Lrelu or a DVE
  single-read bias-move followed by an SBUF-side leaky):
    ACT:  h0, h1 pair (merged, bias-free), h3c0, h2c0 every 3rd tile, head g1
    DVE:  bias-moves for h2c1/h3c1 (+h2c0 usually), 4x-mode bf16 leakys,
          head bias adds
    Pool: both 17-node sum-pools as 5-step scalar_tensor_tensor add-trees
- head: bh1 via ACT bias; bh2 via pre-broadcast [128, L] tensor_tensor adds
  on DVE (no PE bias matmuls). Head tiles are emitted with 2 macro-tiles of
  slack so PE never waits on freshly pooled sums.
"""

import numpy as np
import ml_dtypes

import concourse.bass as bass
import concourse.mybir as mybir
import concourse.tile as tile
from concourse import bacc
from concourse.bass_utils import run_bass_kernel_spmd

F32 = mybir.dt.float32
BF16 = mybir.dt.bfloat16
LRELU = mybir.ActivationFunctionType.Lrelu
ADD = mybir.AluOpType.add
MULT = mybir.AluOpType.mult
MAX = mybir.AluOpType.max
AX_X = mybir.AxisListType.X

NCORES = 8
B, C, T, F, H, L = 16384, 17, 80, 64, 256, 1024
BS = B // NCORES      # graphs per core
R = BS * C            # real node rows per core
MT_G = 30             # graphs per macro-tile
MT_R = MT_G * C       # 510 rows (fits a 2KB PSUM bank)
N_MT = 69             # ceil(BS / MT_G); last tile is 22 graphs of zero pad
G_PAD = N_MT * MT_G   # 2070
R_PAD = G_PAD * C     # 35190
HT_G = 256            # graphs per head tile
N_HT = BS // HT_G     # 8
XL_MT = 4             # macro-tiles per x DMA
HEAD_SLACK_G = 2 * MT_G  # delay head emission so pooled sums are ready
H2C0_ACT_EVERY = 3    # every Nth macro-tile h2c0 runs on ACT instead of DVE
SLOPE = 0.01

_CACHE = {}


def _leaky_np(v):
    return np.where(v > 0, v, SLOPE * v)


def _build(reps=1):
    nc = bacc.Bacc("TRN2", target_bir_lowering=False, debug=False)

    xt_p = nc.declare_dram_parameter("xt", [T + 1, R_PAD], BF16, isOutput=False)
    wembx_p = nc.declare_dram_parameter("wembx", [T + 1, F], BF16, isOutput=False)
    w1x_p = nc.declare_dram_parameter("w1x", [F + 1, H], BF16, isOutput=False)
    w2_p = nc.declare_dram_parameter("w2", [128, 2, H], BF16, isOutput=False)
    b2_p = nc.declare_dram_parameter("b2", [128, 2], F32, isOutput=False)
    w3_p = nc.declare_dram_parameter("w3", [128, 2, H], BF16, isOutput=False)
    b3_p = nc.declare_dram_parameter("b3", [128, 2], F32, isOutput=False)
    wh1_p = nc.declare_dram_parameter("wh1", [128, 2, H], BF16, isOutput=False)
    bh1_p = nc.declare_dram_parameter("bh1", [128, 2], F32, isOutput=False)
    wh2_p = nc.declare_dram_parameter("wh2", [128, 2, L], BF16, isOutput=False)
    bh2b_p = nc.declare_dram_parameter("bh2b", [128, L], F32, isOutput=False)
    out_p = nc.declare_dram_parameter("out", [BS, L], F32, isOutput=True)

    with tile.TileContext(nc) as tc:
        with tc.tile_pool(name="consts", bufs=1) as consts:
            wembx_t = consts.tile([T + 1, F], BF16)
            w1x_t = consts.tile([F + 1, H], BF16)
            w2_t = consts.tile([128, 2, H], BF16)
            b2_t = consts.tile([128, 2], F32)
            w3_t = consts.tile([128, 2, H], BF16)
            b3_t = consts.tile([128, 2], F32)
            wh1_t = consts.tile([128, 2, H], BF16)
            bh1_t = consts.tile([128, 2], F32)
            wh2_t = consts.tile([128, 2, L], BF16)
            bh2b_t = consts.tile([128, L], F32)
            for dst_t, src_p in [
                (wembx_t, wembx_p), (w1x_t, w1x_p),
                (w2_t, w2_p), (b2_t, b2_p), (w3_t, w3_p), (b3_t, b3_p),
                (wh1_t, wh1_p), (bh1_t, bh1_p),
                (wh2_t, wh2_p), (bh2b_t, bh2b_p),
            ]:
                nc.sync.dma_start(dst_t[:], src_p[:])

            # pooled per-graph sums (head input), persistent
            gt = consts.tile([128, 2, G_PAD], BF16)

            # constant 0.01 tile: Pool-side leaky slope operand
            c001 = consts.tile([128, MT_R], BF16)
            nc.vector.memset(c001[:], SLOPE)

            # h0 tiles carry a constant-ones row 64 so conv1's bias rides the
            # matmul; manual 3-buffer rotation keeps the row intact.
            ones_row = consts.tile([1, MT_R], F32)
            nc.vector.memset(ones_row[:], 1.0)
            h0s = []
            for i in range(3):
                h0buf = consts.tile([F + 1, MT_R], BF16, name=f"h0_{i}")
                nc.vector.tensor_copy(h0buf[F:F + 1, :], ones_row[:])
                h0s.append(h0buf)

            for _rep in range(reps):
              with tc.tile_pool(name="xl", bufs=2) as xlp, \
                 tc.tile_pool(name="hw", bufs=3) as hw, \
                 tc.tile_pool(name="hd", bufs=2) as hd, \
                 tc.tile_pool(name="scr", bufs=2) as scr, \
                 tc.tile_pool(name="ps0", bufs=1, space="PSUM") as ps0p, \
                 tc.tile_pool(name="pspair", bufs=3, space="PSUM") as pppool, \
                 tc.tile_pool(name="pso", bufs=1, space="PSUM") as psop:

                def emit_head(ht):
                    g0 = ht * HT_G
                    g1 = hd.tile([128, 2, HT_G], BF16, tag="g1")
                    for m in range(2):
                        psg1 = ps0p.tile([128, 512], F32, tag="ps0", name="psg1")[:, 0:HT_G]
                        nc.tensor.matmul(psg1[:],
                                         wh1_t[:, 0, m * 128:(m + 1) * 128],
                                         gt[:, 0, g0:g0 + HT_G],
                                         start=True, stop=False)
                        nc.tensor.matmul(psg1[:],
                                         wh1_t[:, 1, m * 128:(m + 1) * 128],
                                         gt[:, 1, g0:g0 + HT_G],
                                         start=False, stop=True)
                        nc.scalar.activation(g1[:, m, :], psg1[:], LRELU,
                                             bias=bh1_t[:, m:m + 1], scale=1.0,
                                             alpha=SLOPE)
                    for m in range(2):
                        o_sb = hd.tile([128, L], F32, tag="osb")
                        for nb in range(2):
                            pso = psop.tile([128, 512], F32, tag="pso")
                            nc.tensor.matmul(
                                pso[:],
                                g1[:, 0, m * 128:(m + 1) * 128],
                                wh2_t[:, 0, nb * 512:(nb + 1) * 512],
                                start=True, stop=False)
                            nc.tensor.matmul(
                                pso[:],
                                g1[:, 1, m * 128:(m + 1) * 128],
                                wh2_t[:, 1, nb * 512:(nb + 1) * 512],
                                start=False, stop=True)
                            nc.vector.tensor_tensor(
                                o_sb[:, nb * 512:(nb + 1) * 512], pso[:],
                                bh2b_t[:, nb * 512:(nb + 1) * 512], op=ADD)
                        nc.sync.dma_start(
                            out_p[g0 + m * 128:g0 + (m + 1) * 128, :], o_sb[:])

                def dve_move(pp_t, c, b_t, utag):
                    # single-PSUM-read bias add into bf16 SBUF
                    u = scr.tile([128, MT_R], BF16, tag=utag, name="u")
                    nc.vector.tensor_scalar(u[:], pp_t[:, c, 0:MT_R],
                                            b_t[:, c:c + 1], None, op0=ADD)
                    return u

                def phase2(mt, h2):
                    # conv3 [256]->[256] of the PREVIOUS macro-tile: issued a
                    # full tile late so the cross-engine h2 chain is done and
                    # no in-order queue ever stalls on same-tile products
                    pp3 = pppool.tile([128, 2, 512], F32, tag="pp")
                    for c in (1, 0):
                        nc.tensor.matmul(pp3[:, c, 0:MT_R],
                                         w3_t[:, 0, c * 128:(c + 1) * 128],
                                         h2[:, 0, :], start=True, stop=False)
                        nc.tensor.matmul(pp3[:, c, 0:MT_R],
                                         w3_t[:, 1, c * 128:(c + 1) * 128],
                                         h2[:, 1, :], start=False, stop=True)
                    h3 = hw.tile([128, 2, MT_R], BF16, tag="h3")
                    u3 = dve_move(pp3, 1, b3_t, "u3")
                    v3 = scr.tile([128, MT_R], BF16, tag="v3")
                    nc.gpsimd.tensor_tensor(v3[:], u3[:], c001[:], op=MULT)
                    nc.scalar.activation(h3[:, 0, :], pp3[:, 0, 0:MT_R],
                                         LRELU, bias=b3_t[:, 0:1],
                                         scale=1.0, alpha=SLOPE)
                    nc.vector.tensor_tensor(h3[:, 1, :], u3[:], v3[:], op=MAX)

                    # 17-node sum pool (1/17 folded into Wh1): Pool halves
                    # nodes 0..15, DVE reduces the 8 sums, adds node 16
                    g0 = mt * MT_G
                    h3v = h3[:, :, :].rearrange("p c (g s) -> p c g s", s=C)
                    sc8 = scr.tile([128, 2, MT_G, 8], F32, tag="sc8")
                    tmp = scr.tile([128, 2, MT_G], F32, tag="tmp")
                    nc.gpsimd.tensor_tensor(sc8[:, :, :, :],
                                            h3v[:, :, :, 0:8],
                                            h3v[:, :, :, 8:16], op=ADD)
                    with nc.allow_low_precision(
                            reason="pooled sums rounded to bf16 for the "
                                   "bf16 head matmul"):
                        nc.vector.tensor_reduce(
                            out=tmp[:, :, :], in_=sc8[:, :, :, :],
                            op=ADD, axis=AX_X)
                        nc.vector.tensor_tensor(
                            gt[:, :, g0:g0 + MT_G], tmp[:, :, :],
                            h3v[:, :, :, 16:17].rearrange(
                                "p c g o -> p c (g o)"), op=ADD)

                next_ht = 0
                xl = None
                prev = None
                for mt in range(N_MT):
                    j = mt % XL_MT
                    if j == 0:
                        cols = min(XL_MT * MT_R, R_PAD - mt * MT_R)
                        xl = xlp.tile([T + 1, XL_MT * MT_R], BF16, tag="xl")
                        nc.sync.dma_start(
                            xl[:, 0:cols],
                            xt_p[:, mt * MT_R:mt * MT_R + cols])
                    xs = xl[:, j * MT_R:(j + 1) * MT_R]

                    # embed [81]->[64] (bias via ones row), leaky on ACT
                    ps0 = ps0p.tile([128, 512], F32, tag="ps0", name="ps0")[0:F, 0:MT_R]
                    nc.tensor.matmul(ps0[:], wembx_t[:], xs,
                                     start=True, stop=True)
                    h0 = h0s[mt % 3]
                    nc.scalar.activation(h0[0:F, :], ps0[:], LRELU,
                                         bias=0.0, scale=1.0, alpha=SLOPE)

                    # conv1 [65]->[256] (bias via h0 ones row), merged pair
                    # leaky on ACT
                    pp1 = pppool.tile([128, 2, 512], F32, tag="pp")
                    for c in range(2):
                        nc.tensor.matmul(pp1[:, c, 0:MT_R],
                                         w1x_t[:, c * 128:(c + 1) * 128],
                                         h0[:], start=True, stop=True)
                    h1 = hw.tile([128, 2, MT_R], BF16, tag="h1")
                    nc.scalar.activation(h1[:, :, :], pp1[:, :, 0:MT_R],
                                         LRELU, bias=0.0, scale=1.0,
                                         alpha=SLOPE)

                    # conv2 [256]->[256]; c1 first so its DVE->Pool->DVE
                    # leaky chain starts as early as possible
                    pp2 = pppool.tile([128, 2, 512], F32, tag="pp")
                    for c in (1, 0):
                        nc.tensor.matmul(pp2[:, c, 0:MT_R],
                                         w2_t[:, 0, c * 128:(c + 1) * 128],
                                         h1[:, 0, :], start=True, stop=False)
                        nc.tensor.matmul(pp2[:, c, 0:MT_R],
                                         w2_t[:, 1, c * 128:(c + 1) * 128],
                                         h1[:, 1, :], start=False, stop=True)
                    h2 = hw.tile([128, 2, MT_R], BF16, tag="h2")
                    u2 = dve_move(pp2, 1, b2_t, "u2")
                    v2 = scr.tile([128, MT_R], BF16, tag="v2")
                    nc.gpsimd.tensor_tensor(v2[:], u2[:], c001[:], op=MULT)
                    nc.scalar.activation(h2[:, 0, :], pp2[:, 0, 0:MT_R],
                                         LRELU, bias=b2_t[:, 0:1],
                                         scale=1.0, alpha=SLOPE)
                    nc.vector.tensor_tensor(h2[:, 1, :], u2[:], v2[:], op=MAX)

                    if prev is not None:
                        phase2(*prev)
                        done = (prev[0] + 1) * MT_G - HEAD_SLACK_G
                        while (next_ht < N_HT
                               and (next_ht + 1) * HT_G <= done):
                            emit_head(next_ht)
                            next_ht += 1
                    prev = (mt, h2)

                phase2(*prev)
                while next_ht < N_HT:
                    emit_head(next_ht)
                    next_ht += 1

    nc.compile()
    return nc


def _get_nc(reps=1):
    key = ("nc", reps)
    if key not in _CACHE:
        _CACHE[key] = _build(reps)
    return _CACHE[key]


def prep_in_maps(x, W_emb, b_emb, W1, b1, W2, b2, W3, b3, Wh1, bh1, Wh2, bh2):
    """Host-side layout prep: per-core input maps for run_bass_kernel_spmd."""
    bf16 = ml_dtypes.bfloat16

    def kchunks(w):
        # [256, out] -> [128, 2, out] (k-chunk as middle axis)
        return np.ascontiguousarray(
            w.reshape(2, 128, w.shape[1]).transpose(1, 0, 2)).astype(bf16)

    def bcols(b):
        # [256] -> [128, 2]
        return np.ascontiguousarray(b.reshape(2, 128).T)

    weights = {
        "wembx": np.ascontiguousarray(
            np.concatenate([W_emb, b_emb[None, :]], axis=0)).astype(bf16),
        "w1x": np.ascontiguousarray(
            np.concatenate([W1, b1[None, :]], axis=0)).astype(bf16),
        "w2": kchunks(W2), "b2": bcols(b2),
        "w3": kchunks(W3), "b3": bcols(b3),
        "wh1": kchunks(Wh1 * (1.0 / C)), "bh1": bcols(bh1),
        "wh2": kchunks(Wh2),
        "bh2b": np.ascontiguousarray(
            np.broadcast_to(bh2[None, :], (128, L)).copy()),
    }

    xr = x.reshape(B * C, T)
    in_maps = []
    for i in range(NCORES):
        xt = np.zeros((T + 1, R_PAD), np.float32)
        xt[0:T, 0:R] = xr[i * R:(i + 1) * R].T
        xt[T, :] = 1.0
        m = dict(weights)
        m["xt"] = xt.astype(bf16)
        in_maps.append(m)
    return in_maps


def _fixup_graph0(x, W_emb, b_emb, W1, b1, W2, b2, W3, b3, Wh1, bh1, Wh2, bh2,
                  src, dst):
    """Exact recompute of batch element 0 with real GCN propagation."""
    deg = np.ones(C, np.float64)
    np.add.at(deg, dst.astype(np.int64), 1.0)
    dinv = 1.0 / np.sqrt(deg)
    A = np.zeros((C, C), np.float64)
    A[np.arange(C), np.arange(C)] = dinv * dinv
    np.add.at(A, (dst.astype(np.int64), src.astype(np.int64)),
              dinv[src.astype(np.int64)] * dinv[dst.astype(np.int64)])

    h = _leaky_np(x[0].astype(np.float64) @ W_emb + b_emb)
    for Wc, bc in [(W1, b1), (W2, b2), (W3, b3)]:
        h = _leaky_np(A @ (h @ Wc) + bc)
    g = h.mean(axis=0)
    return (_leaky_np(g @ Wh1 + bh1) @ Wh2 + bh2).astype(np.float32)


def kernel(x, W_emb, b_emb, W1, b1, W2, b2, W3, b3, Wh1, bh1, Wh2, bh2,
           src, dst):
    x = np.ascontiguousarray(np.asarray(x, np.float32))
    W_emb = np.asarray(W_emb, np.float32)
    b_emb = np.asarray(b_emb, np.float32)
    W1 = np.asarray(W1, np.float32)
    b1 = np.asarray(b1, np.float32)
    W2 = np.asarray(W2, np.float32)
    b2 = np.asarray(b2, np.float32)
    W3 = np.asarray(W3, np.float32)
    b3 = np.asarray(b3, np.float32)
    Wh1 = np.asarray(Wh1, np.float32)
    bh1 = np.asarray(bh1, np.float32)
    Wh2 = np.asarray(Wh2, np.float32)
    bh2 = np.asarray(bh2, np.float32)

    in_maps = prep_in_maps(x, W_emb, b_emb, W1, b1, W2, b2, W3, b3,
                           Wh1, bh1, Wh2, bh2)

    nc = _get_nc()
    res = run_bass_kernel_spmd(nc, in_maps, core_ids=list(range(NCORES)))
    out = np.concatenate([res.results[i]["out"] for i in range(NCORES)], axis=0)

    out[0] = _fixup_graph0(x, W_emb, b_emb, W1, b1, W2, b2, W3, b3,
                           Wh1, bh1, Wh2, bh2, np.asarray(src), np.asarray(dst))
    return out
